# revision 1
# baseline (speedup 1.0000x reference)
"""Trainium2 Bass kernel for the non-local attention block (nn_ASM_5196910428634).

8 NeuronCores, data-parallel over batch (1 element per core).  Per core:
  x = fuse[b] as [C=256, HW=4096]
  theta = Wt @ x + bt                [128, 4096]   (f32r conv, fp16 output)
  phi   = pool2(Wp @ x + bp)         [128, 1024]   (f32r conv, fp16 output)
  g     = pool2(Wg @ x + bg)         [128, 1024]   (f32r conv)
  fT[k, n] = sum_ci phi[ci, k] theta[ci, n]        (fp16 matmul, k=1024)
  A = exp(fT)  -> bf16               (softmax w/o max-subtraction: |f| << 88,
                                      and bf16 carries the fp32 exponent range)
  sums[n] = sum_k A[k, n]            (ones-vector bf16 matmul on PE)
  yT[c, n] = sum_k gT[k, c] A[k, n]  (bf16 matmul, unnormalized)
  y_norm = yT * (1/sums)[broadcast]  (K=1 bf16 broadcast matmul +
                                      reciprocal_approx_fast, fused into the
                                      PSUM->SBUF copy)
  z = (WW @ y_norm) * inv + (x + bias2)   (bf16 W conv; BN folded on host:
                                           inv = gamma/sqrt(var+eps),
                                           bias2 = (Wb - mean)*inv + beta)
Host assembles out = concat([lc, z, gc], axis=1) (pure pass-through channels).

Precision strategy: the f path (convs for theta/phi and the fT matmul) is the
exp-amplified one - convs run float32r (TF32-class, 2 cyc/row), theta/phi are
rounded to fp16 (4x tighter mantissa than bf16, range fits |f|<=~60).  The
post-softmax path (y, sums, W conv) tolerates bf16 (1 cyc/row).  End-to-end
absmax relative error vs the fp32 jax reference: ~3.9e-3.

Schedule: x is DMAd as f32r straight from DRAM (f32r bits are fp32 bits; the
PE rounds internally); weights are host-packed into one contiguous array and
ride the two HWDGE queues ahead of the 4MB x stream in consumption order (phi
chunks first); the three projections are interleaved q-major so the PE chases
the x DMA stream; the 8 attention windows of 512 columns are software-
pipelined in three phases - f/exp of window w, sums/y/normalize of w-1, and
Wconv/BN/store of w-2 - so the PE never waits on the ScalarE exp or the DVE
reciprocal chain (the last two f-pairs of window w are emitted after
consume_a(w-1) so the psf-slot wait on exp is covered by sums/y matmuls); column sums use an all-ones 128x128 stationary matmul whose
output rows ARE the broadcast, so 1/sums needs no separate broadcast step;
PSUM is packed as psf[128,1024]x2 + psy[128,512]x2 + pss[128,512]x2 = 8 banks.

Measured on 8 axon-tunneled trn2 cores: ~92.5us mean / ~94us max per-core HW
exec time in a quiet chip epoch (NTFF total_time, includes the ~7us engine
preamble and ~11us tail drain+barrier; under sustained-power P0 downclock the
same kernel reads ~108us), absmax relative error 3.9e-3.  First matmul issues
at ~12us: the phi weights and the first 512-column halves of x lead both HWDGE
queues so the PE's first dependency is only ~0.6MB of DMA.
"""

import numpy as np

import concourse.bass as bass
import concourse.tile as tile
from concourse import bacc, mybir
from concourse.bass_utils import run_bass_kernel_spmd
from concourse.masks import make_identity

F32 = mybir.dt.float32
F32R = mybir.dt.float32r
BF16 = mybir.dt.bfloat16
FP16 = mybir.dt.float16
AX = mybir.AluOpType
AF = mybir.ActivationFunctionType

B, C, HW = 8, 256, 4096
CI = 128
NK = 1024
N_CORES = 8
BN_EPS = 1e-5

WIN = 512
NWIN = HW // WIN


def build_program():
    nc = bacc.Bacc("TRN2", target_bir_lowering=False, debug=False,
                   num_devices=N_CORES)

    x_d = nc.dram_tensor("x", [C, HW], F32R, kind="ExternalInput").ap()
    wpk_d = nc.dram_tensor("wpk", [128, 8 * 128], F32R, kind="ExternalInput").ap()
    bpk_d = nc.dram_tensor("bpk", [128, 7], F32, kind="ExternalInput").ap()
    z_d = nc.dram_tensor("z", [C, HW], F32, kind="ExternalOutput").ap()

    with tile.TileContext(nc) as tc:
        with (
            tc.tile_pool(name="const", bufs=1) as consts,
            tc.tile_pool(name="xs", bufs=1) as xs,
            tc.tile_pool(name="big", bufs=1) as big,
            tc.tile_pool(name="stage", bufs=2) as stage,
            tc.tile_pool(name="outp", bufs=3) as outp,
            tc.tile_pool(name="pp", bufs=1, space="PSUM") as pp,
        ):
            ident = consts.tile([128, 128], F32, tag="ident", name="ident")
            make_identity(nc, ident)

            # ---------------- loads ----------------
            # x first (it is the long pole), issued from several engines so
            # descriptor issue does not serialize behind one queue
            x_r = [[None] + [xs.tile([128, 1024], F32R, tag=f"xr{t}{q}",
                                     name=f"xr{t}{q}") for q in range(1, 4)]
                   for t in range(2)]
            x0h = [[xs.tile([128, 512], F32R, tag=f"x0h{t}{u}",
                            name=f"x0h{t}{u}") for u in range(2)]
                   for t in range(2)]
            w_r = consts.tile([128, 8, 128], F32R, tag="wr", name="wr")
            bpk = consts.tile([128, 7], F32, tag="bpk", name="bpk")
            biases = bpk[:, 0:3]
            binv = bpk[:, 3:5]
            bb2 = bpk[:, 5:7]

            # load order tuned so the first conv's deps land earliest:
            # scalar queue: theta/phi/g weights (384KB), then x upper half;
            # sync queue: x lower half first; the W-conv weights (not needed
            # until the first consume_b, ~30us in) ride behind.
            wfl = w_r.rearrange("p j c -> p (j c)")
            # phi weights + the first 512-column halves of x lead both queues
            # so the very first conv matmul waits on only ~0.6MB
            nc.scalar.dma_start(out=wfl[:, 2 * 128:4 * 128],
                                in_=wpk_d[:, 2 * 128:4 * 128])
            nc.sync.dma_start(out=x0h[0][0], in_=x_d[0:128, 0:512])
            nc.scalar.dma_start(out=x0h[1][0], in_=x_d[128:256, 0:512])
            nc.sync.dma_start(out=x0h[0][1], in_=x_d[0:128, 512:1024])
            nc.scalar.dma_start(out=x0h[1][1], in_=x_d[128:256, 512:1024])
            nc.sync.dma_start(out=wfl[:, 0:2 * 128], in_=wpk_d[:, 0:2 * 128])
            nc.scalar.dma_start(out=wfl[:, 4 * 128:6 * 128],
                                in_=wpk_d[:, 4 * 128:6 * 128])
            nc.sync.dma_start(out=x_r[0][1], in_=x_d[0:128, 1024:2048])
            nc.scalar.dma_start(out=bpk, in_=bpk_d)
            nc.scalar.dma_start(out=x_r[1][1], in_=x_d[128:256, 1024:2048])
            nc.sync.dma_start(out=wfl[:, 6 * 128:], in_=wpk_d[:, 6 * 128:])
            for q in range(2, 4):
                nc.sync.dma_start(out=x_r[0][q],
                                  in_=x_d[0:128, q * 1024:(q + 1) * 1024])
                nc.scalar.dma_start(out=x_r[1][q],
                                    in_=x_d[128:256, q * 1024:(q + 1) * 1024])

            def xr_ap(t, q, u):
                if q == 0:
                    return x0h[t][u]
                return x_r[t][q][:, u * 512:u * 512 + 512]
            ones_mat = consts.tile([128, 128], BF16, tag="ones_mat",
                                    name="ones_mat")
            nc.vector.memset(ones_mat, 1.0)
            # ---------------- projections ----------------
            theta_r = big.tile([128, HW], FP16, tag="theta", name="theta")
            pf_phi = big.tile([128, 64, 64], F32, tag="pf_phi", name="pf_phi")
            pf_g = big.tile([128, 64, 64], F32, tag="pf_g", name="pf_g")
            m1 = big.tile([128, 64, 32], F32, tag="m1", name="m1")
            phi_r = big.tile([128, NK], FP16, tag="phi", name="phi")
            g_pool = big.tile([128, NK], F32, tag="gpool", name="gpool")
            gT_r = big.tile([128, 8, 128], BF16, tag="gT", name="gT")

            pf_phi_f = pf_phi.rearrange("p h w -> p (h w)")
            pf_g_f = pf_g.rearrange("p h w -> p (h w)")
            m1g = big.tile([128, 64, 32], F32, tag="m1g", name="m1g")

            # bf16 weights for the W conv (cheap cast; W path tolerates bf16)
            wW_bf = consts.tile([128, 2, 128], BF16, tag="wWbf", name="wWbf")
            for o in range(2):
                nc.vector.tensor_copy(wW_bf[:, o, :], w_r[:, 6 + o, :])

            def pool1(srcf, dst_m1, q):
                # first maxpool pass (w-pairs) for the 16 rows filled by q
                a = srcf.rearrange("p h (w2 two) -> p h w2 two", two=2)
                nc.vector.tensor_max(dst_m1[:, 16 * q:16 * (q + 1), :],
                                     a[:, 16 * q:16 * (q + 1), :, 0],
                                     a[:, 16 * q:16 * (q + 1), :, 1])

            def pool2(src_m1, dst):
                b_ = src_m1.rearrange("p (h2 two) w -> p h2 two w", two=2)
                nc.vector.tensor_max(
                    dst.rearrange("p (h w) -> p h w", h=32),
                    b_[:, :, 0, :], b_[:, :, 1, :])

            # convs interleaved q-major so the PE chases the x DMA stream;
            # phi first (its pooled result gates the first attention window)
            conv_dsts = [(1, pf_phi_f), (0, theta_r), (2, pf_g_f)]
            for q in range(4):
                for widx, dst in conv_dsts:
                    psc = pp.tile([128, 1024], F32, tag="psf", name="psc",
                                  bufs=2)
                    for u in range(2):
                        for t in range(2):
                            nc.tensor.matmul(
                                psc[:, u * 512:u * 512 + 512],
                                w_r[:, 2 * widx + t, :],
                                xr_ap(t, q, u),
                                start=(t == 0), stop=(t == 1))
                    nc.scalar.activation(
                        out=dst[:, q * 1024:(q + 1) * 1024], in_=psc,
                        func=AF.Identity, bias=biases[:, widx:widx + 1])
                pool1(pf_phi, m1, q)
                pool1(pf_g, m1g, q)

            pool2(m1, phi_r)

            # ---------------- attention (software-pipelined windows) -------
            a_tiles = [None] * NWIN

            def produce(w, k2r=range(4)):
                # fT = phi^T theta for window w, then exp -> A[w]
                if a_tiles[w] is None:
                    a_tiles[w] = big.tile([128, 8, WIN], BF16, tag="A",
                                          name=f"A{w}", bufs=2)
                a_t = a_tiles[w]
                sl = slice(w * WIN, (w + 1) * WIN)
                for k2 in k2r:
                    psf = pp.tile([128, 2 * WIN], F32, tag="psf", name="psf",
                                  bufs=2)
                    for j in range(2):
                        nc.tensor.matmul(
                            psf[:, j * WIN:(j + 1) * WIN],
                            phi_r[:, (2 * k2 + j) * 128:(2 * k2 + j + 1) * 128],
                            theta_r[:, sl], start=True, stop=True)
                    nc.scalar.activation(
                        out=a_t.rearrange("p k n -> p (k n)")
                        [:, 2 * k2 * WIN:(2 * k2 + 2) * WIN],
                        in_=psf, func=AF.Exp)

            y_tiles = [None] * NWIN

            def consume_a(w):
                a_t = a_tiles[w]
                # column sums first (the reciprocal chain hangs off them),
                # then yT accumulation
                # all-ones stationary: every psum row = column sums, so the
                # partition-broadcast of 1/sums comes for free
                pss = pp.tile([128, WIN], F32, tag="pss", name="pss", bufs=2)
                for k in range(8):
                    nc.tensor.matmul(pss, ones_mat, a_t[:, k, :],
                                     start=(k == 0), stop=(k == 7))
                psy = pp.tile([128, WIN], F32, tag="psy", name="psy", bufs=2)
                for k in range(8):
                    nc.tensor.matmul(psy, gT_r[:, k, :], a_t[:, k, :],
                                     start=(k == 0), stop=(k == 7))
                rbc = stage.tile([128, WIN], F32, tag="rbc", name="rbc")
                nc.vector.reciprocal_approx_fast(out=rbc, in_=pss)
                # y_norm = yT * rbc  (fused into the PSUM->SBUF copy)
                y_r = stage.tile([128, WIN], BF16, tag="yr", name="yr")
                y_tiles[w] = y_r
                nc.vector.scalar_tensor_tensor(out=y_r, in0=psy, scalar=1.0,
                                               in1=rbc, op0=AX.mult,
                                               op1=AX.mult)

            def consume_b(w):
                # W conv + BN + residual, one window behind consume_a so the
                # DVE normalize chain never stalls the PE
                y_r = y_tiles[w]
                base = w * WIN
                q, r5 = base // 1024, base % 1024
                for o in range(2):
                    psW = pp.tile([128, WIN], F32, tag="psy", name="psW", bufs=2)
                    nc.tensor.matmul(psW, wW_bf[:, o, :], y_r,
                                     start=True, stop=True)
                    t1 = outp.tile([128, WIN], F32, tag="t1", name="t1")
                    nc.vector.scalar_tensor_tensor(
                        out=t1, in0=psW, scalar=binv[:, o:o + 1],
                        in1=x_b[o][q][:, r5:r5 + WIN],
                        op0=AX.mult, op1=AX.add)
                    nc.sync.dma_start(
                        out=z_d[o * 128:(o + 1) * 128, base:base + WIN], in_=t1)

            produce(0)
            # g path + residual prep overlap the first window's f/exp
            pool2(m1g, g_pool)
            for k in range(8):
                ptr = pp.tile([128, 512], F32, tag="psy", name="ptr", bufs=2)
                nc.tensor.transpose(ptr[:, :128],
                                    g_pool[:, k * 128:(k + 1) * 128], ident)
                nc.vector.tensor_copy(gT_r[:, k, :], ptr[:, :128])
            x_b = [[xs.tile([128, 1024], F32, tag=f"xb{t}{q}", name=f"xb{t}{q}")
                    for q in range(4)] for t in range(2)]
            for t in range(2):
                for u in range(2):
                    nc.vector.tensor_scalar_add(x_b[t][0][:, u * 512:u * 512 + 512],
                                                x0h[t][u].bitcast(F32),
                                                bb2[:, t:t + 1])
                for q in range(1, 4):
                    nc.vector.tensor_scalar_add(x_b[t][q],
                                                x_r[t][q].bitcast(F32),
                                                bb2[:, t:t + 1])
            produce(1)
            consume_a(0)
            for w in range(2, NWIN):
                produce(w, range(0, 2))
                consume_b(w - 2)
                consume_a(w - 1)
                produce(w, range(2, 4))
            consume_b(NWIN - 2)
            consume_a(NWIN - 1)
            consume_b(NWIN - 1)
    nc.compile()
    return nc


_nc_cache = None


def _get_nc():
    global _nc_cache
    if _nc_cache is None:
        _nc_cache = build_program()
    return _nc_cache


def run(inputs, trace=False, **kw):
    lc = np.asarray(inputs["lc"], dtype=np.float32)
    fuse = np.asarray(inputs["fuse"], dtype=np.float32)
    gc = np.asarray(inputs["gc"], dtype=np.float32)

    inv = np.asarray(inputs["bn_gamma"], np.float32) / np.sqrt(
        np.asarray(inputs["bn_var"], np.float32) + BN_EPS)
    bias2 = ((np.asarray(inputs["W_b"], np.float32)
              - np.asarray(inputs["bn_mean"], np.float32)) * inv
             + np.asarray(inputs["bn_beta"], np.float32))

    # pack weights: wpk[p, j*128 + c] = Wchunk_j[p, c] where chunks 0..5 are
    # the two contraction halves of theta/phi/g (transposed weights) and
    # chunks 6..7 are the two output halves of the W conv
    wpk = np.empty((128, 8 * 128), np.float32)
    for i, nm in enumerate(("theta_w", "phi_w", "g_w")):
        wt = np.asarray(inputs[nm], np.float32).T.reshape(2, 128, 128)
        wpk[:, 2 * i * 128:(2 * i + 2) * 128] = \
            wt.transpose(1, 0, 2).reshape(128, 256)
    wpk[:, 6 * 128:] = np.asarray(inputs["W_w"], np.float32).T.reshape(128, 256)
    bpk = np.empty((128, 7), np.float32)
    bpk[:, 0] = np.asarray(inputs["theta_b"], np.float32)
    bpk[:, 1] = np.asarray(inputs["phi_b"], np.float32)
    bpk[:, 2] = np.asarray(inputs["g_b"], np.float32)
    bpk[:, 3:5] = inv.reshape(2, 128).T
    bpk[:, 5:7] = bias2.reshape(2, 128).T
    common = {"wpk": wpk, "bpk": bpk}
    in_maps = []
    for b in range(B):
        m = dict(common)
        m["x"] = np.ascontiguousarray(fuse[b].reshape(C, HW))
        in_maps.append(m)

    nc = _get_nc()
    res = run_bass_kernel_spmd(nc, in_maps, core_ids=list(range(N_CORES)),
                               trace=trace, **kw)

    out = np.empty((B, 3 * C, 64, 64), dtype=np.float32)
    out[:, :C] = lc
    for b in range(B):
        out[b, C:2 * C] = res.results[b]["z"].reshape(C, 64, 64)
    out[:, 2 * C:] = gc
    return out, res


def kernel(**inputs) -> np.ndarray:
    out, _ = run(inputs, trace=False)
    return out

